# revision 1
# baseline (speedup 1.0000x reference)
"""GRU decoder kernel for 8 Trainium2 NeuronCores.

Strategy (model-parallel over output features, weights resident in SBUF):
  - Each core owns a 256-row slice of H (per gate) and a 128-row slice of I.
  - Algebraic fusion: x_t = h_t @ W_fc.T + b_fc feeds gi = x_t @ W_ih.T, so
    with W_comb = W_ih @ W_fc the r,z gates become a single K=2048 matmul
    with W_rz = (W_comb + W_hh)[r,z rows]; i_n uses W_comb[n], h_n uses
    W_hh[n].  Only h needs an 8-way AllGather each step; the fc output is
    written locally to y.  Step 0 runs the unfused form (x_0 is an input).
  - Matmuls run in fp32r (fp32 with 11-bit mantissa, full-rate on the PE);
    all operands are pre-rounded (host or DVE output rounding).
  - The batch (512) is processed as two 256-column halves (keeps the PE at
    full fp32r rate and lets vector work overlap matmuls); both halves feed
    ONE AllGather per step, since the collective's fixed cost (~0.1 ms in
    this environment) dominates its wire time.
"""

import numpy as np

import concourse.mybir as mybir
import concourse.tile as tile
from concourse import bacc
from concourse.bass_utils import run_bass_kernel_spmd

F32 = mybir.dt.float32
F32R = mybir.dt.float32r
AF = mybir.ActivationFunctionType

G = 8          # cores
B = 512        # batch
I = 1024       # input/output feature dim
H = 2048       # hidden dim
HL = H // G    # 256 hidden rows per core (per gate)
XL = I // G    # 128 fc output rows per core
NH = B // 2    # 256 batch half
KH = H // 128  # 16 k-tiles over H
KI = I // 128  # 8 k-tiles over I


def round_fp32r(x: np.ndarray) -> np.ndarray:
    """Round fp32 to fp32r (11-bit mantissa, round-half-to-even).

    Bit-exact with neuronxcc's fp32_to_fp32r and the device's DVE rounding.
    """
    u = np.ascontiguousarray(x, dtype=np.float32).view(np.uint32)
    low = u & np.uint32(0xFFF)
    base = u & np.uint32(0xFFFFF000)
    lsb = (u >> np.uint32(12)) & np.uint32(1)
    add = (low > 0x800) | ((low == 0x800) & (lsb == 1))
    out = base + (add.astype(np.uint32) << np.uint32(12))
    exp = (u >> 23) & np.uint32(0xFF)
    out = np.where(exp == 0xFF, u, out)
    return out.view(np.float32).reshape(np.shape(x))


def build(T: int):
    """Emit the SPMD program for T timesteps."""
    nc = bacc.Bacc("TRN2", target_bir_lowering=False, debug=False, num_devices=G)
    dp = nc.declare_dram_parameter

    w_rz = dp("w_rz", [H, 512], F32R, isOutput=False)    # (Wcomb+Whh)[r|z].T
    w_cn = dp("w_cn", [H, 256], F32R, isOutput=False)    # Wcomb[n].T
    w_hn = dp("w_hn", [H, 256], F32R, isOutput=False)    # Whh[n].T
    w_fc = dp("w_fc", [H, 128], F32R, isOutput=False)    # Wfc[own].T
    w_ih0 = dp("w_ih0", [I, 768], F32R, isOutput=False)  # Wih[r|z|n].T (t=0)
    w_hh0 = dp("w_hh0", [H, 512], F32R, isOutput=False)  # Whh[r|z].T   (t=0)
    x0 = dp("x0", [I, B], F32R, isOutput=False)          # inputs.T
    h0 = dp("h0", [H, B], F32R, isOutput=False)          # h_0.T
    h0_own = dp("h0_own", [HL, B], F32R, isOutput=False)  # own h_0 rows
    b_rz = dp("b_rz", [128, 4], F32, isOutput=False)     # per-tile biases t>=1
    b_rz0 = dp("b_rz0", [128, 4], F32, isOutput=False)   # t=0
    b_in = dp("b_in", [128, 2], F32, isOutput=False)
    b_in0 = dp("b_in0", [128, 2], F32, isOutput=False)
    b_hn = dp("b_hn", [128, 2], F32, isOutput=False)
    b_fc = dp("b_fc", [128, 1], F32, isOutput=False)
    y = dp("y", [T, 128, B], F32, isOutput=True)

    hstage = [nc.dram_tensor(f"hstage{s}", [HL, B], F32R) for s in (0, 1)]
    hgath = [
        nc.dram_tensor(f"hgath{s}", [H, B], F32R, addr_space="Shared")
        for s in (0, 1)
    ]

    with tile.TileContext(nc) as tc:
        with (
            tc.tile_pool(name="weights", bufs=1) as wp,
            tc.tile_pool(name="state", bufs=1) as stp,
            tc.tile_pool(name="scratch", bufs=3) as scr,
            tc.tile_pool(name="w0pool", bufs=3) as w0p,
            tc.tile_pool(name="psum", bufs=6, space="PSUM") as psp,
        ):
            # ---- persistent weights -------------------------------------
            w_rz_sb = wp.tile([128, KH, 512], F32R, tag="w_rz")
            w_cn_sb = wp.tile([128, KH, 256], F32R, tag="w_cn")
            w_hn_sb = wp.tile([128, KH, 256], F32R, tag="w_hn")
            w_fc_sb = wp.tile([128, KH, 128], F32R, tag="w_fc")
            for k0 in range(0, KH, 4):
                sl = slice(k0, k0 + 4)
                nc.sync.dma_start(
                    w_rz_sb[:, sl, :],
                    w_rz[:].rearrange("(k p) m -> p k m", p=128)[:, sl, :],
                )
                nc.sync.dma_start(
                    w_cn_sb[:, sl, :],
                    w_cn[:].rearrange("(k p) m -> p k m", p=128)[:, sl, :],
                )
                nc.sync.dma_start(
                    w_hn_sb[:, sl, :],
                    w_hn[:].rearrange("(k p) m -> p k m", p=128)[:, sl, :],
                )
            nc.sync.dma_start(
                w_fc_sb[:],
                w_fc[:].rearrange("(k p) m -> p k m", p=128),
            )

            # ---- biases --------------------------------------------------
            def bias_tile(param, ncols, tag):
                t = wp.tile([128, ncols], F32, tag=tag)
                nc.sync.dma_start(t[:], param[:])
                return t

            b_rz_sb = bias_tile(b_rz, 4, "b_rz")
            b_rz0_sb = bias_tile(b_rz0, 4, "b_rz0")
            b_in_sb = bias_tile(b_in, 2, "b_in")
            b_in0_sb = bias_tile(b_in0, 2, "b_in0")
            b_hn_sb = bias_tile(b_hn, 2, "b_hn")
            b_fc_sb = bias_tile(b_fc, 1, "b_fc")

            # ---- state: gathered h (ping-pong x half), own h slice ------
            ht_sb = [
                [
                    stp.tile([128, KH, NH], F32R, tag=f"ht{pp}{hf}",
                             name=f"ht{pp}{hf}")
                    for hf in (0, 1)
                ]
                for pp in (0, 1)
            ]
            h_own = [
                [
                    stp.tile([128, 2, NH], F32R, tag=f"ho{pp}{hf}",
                             name=f"ho{pp}{hf}")
                    for hf in (0, 1)
                ]
                for pp in (0, 1)
            ]
            for hf in (0, 1):
                cols = slice(hf * NH, (hf + 1) * NH)
                for k0 in range(0, KH, 8):
                    sl = slice(k0, k0 + 8)
                    nc.sync.dma_start(
                        ht_sb[0][hf][:, sl, :],
                        h0[:].rearrange("(k p) n -> p k n", p=128)[:, sl, cols],
                    )
                nc.sync.dma_start(
                    h_own[0][hf][:],
                    h0_own[:].rearrange("(j p) n -> p j n", p=128)[:, :, cols],
                )

            # ---- time loop ----------------------------------------------
            for t in range(T):
                cur, nxt = t % 2, 1 - (t % 2)
                for hf in (0, 1):
                    cols = slice(hf * NH, (hf + 1) * NH)
                    ht_c = ht_sb[cur][hf]
                    brz = b_rz_sb if t > 0 else b_rz0_sb
                    bin_ = b_in_sb if t > 0 else b_in0_sb

                    ps_r = [None, None]
                    ps_z = [None, None]
                    ps_in = [None, None]
                    if t == 0:
                        # unfused first step: x-part streamed from DRAM
                        for j in (0, 1):
                            ps_r[j] = psp.tile([128, NH], F32, tag="ps", name="ps_r")
                            ps_z[j] = psp.tile([128, NH], F32, tag="ps", name="ps_z")
                            ps_in[j] = psp.tile([128, NH], F32, tag="ps", name="ps_in")
                        wih_r = w_ih0[:].rearrange("(k p) m -> p k m", p=128)
                        x0_r = x0[:].rearrange("(k p) n -> p k n", p=128)
                        for k in range(KI):
                            wt = w0p.tile([128, 768], F32R, tag="w0ih")
                            nc.sync.dma_start(wt[:], wih_r[:, k, :])
                            xt = w0p.tile([128, NH], F32R, tag="w0x")
                            nc.sync.dma_start(xt[:], x0_r[:, k, cols])
                            for j in (0, 1):
                                nc.tensor.matmul(
                                    ps_r[j][:], wt[:, j * 128:(j + 1) * 128],
                                    xt[:], start=(k == 0), stop=False,
                                )
                                nc.tensor.matmul(
                                    ps_z[j][:], wt[:, 256 + j * 128:384 + j * 128],
                                    xt[:], start=(k == 0), stop=False,
                                )
                                nc.tensor.matmul(
                                    ps_in[j][:], wt[:, 512 + j * 128:640 + j * 128],
                                    xt[:], start=(k == 0), stop=(k == KI - 1),
                                )
                        whh_r = w_hh0[:].rearrange("(k p) m -> p k m", p=128)
                        for k in range(KH):
                            wt = w0p.tile([128, 512], F32R, tag="w0hh")
                            nc.sync.dma_start(wt[:], whh_r[:, k, :])
                            for j in (0, 1):
                                nc.tensor.matmul(
                                    ps_r[j][:], wt[:, j * 128:(j + 1) * 128],
                                    ht_c[:, k, :], start=False, stop=(k == KH - 1),
                                )
                                nc.tensor.matmul(
                                    ps_z[j][:], wt[:, 256 + j * 128:384 + j * 128],
                                    ht_c[:, k, :], start=False, stop=(k == KH - 1),
                                )
                    else:
                        for j in (0, 1):
                            ps_r[j] = psp.tile([128, NH], F32, tag="ps", name="ps_r")
                            for k in range(KH):
                                nc.tensor.matmul(
                                    ps_r[j][:],
                                    w_rz_sb[:, k, j * 128:(j + 1) * 128],
                                    ht_c[:, k, :],
                                    start=(k == 0), stop=(k == KH - 1),
                                )
                        for j in (0, 1):
                            ps_z[j] = psp.tile([128, NH], F32, tag="ps", name="ps_z")
                            for k in range(KH):
                                nc.tensor.matmul(
                                    ps_z[j][:],
                                    w_rz_sb[:, k, 256 + j * 128:384 + j * 128],
                                    ht_c[:, k, :],
                                    start=(k == 0), stop=(k == KH - 1),
                                )

                    # nonlinearity chain per j-tile
                    for j in (0, 1):
                        r_t = scr.tile([128, NH], F32, tag="r")
                        nc.scalar.activation(
                            r_t[:], ps_r[j][:], AF.Sigmoid, bias=brz[:, j:j + 1]
                        )
                        z_t = scr.tile([128, NH], F32, tag="z")
                        nc.scalar.activation(
                            z_t[:], ps_z[j][:], AF.Sigmoid, bias=brz[:, 2 + j:3 + j]
                        )
                        if t == 0:
                            in_t = scr.tile([128, NH], F32, tag="in")
                            nc.scalar.activation(
                                in_t[:], ps_in[j][:], AF.Identity,
                                bias=bin_[:, j:j + 1],
                            )
                        else:
                            ps_in[j] = psp.tile([128, NH], F32, tag="ps", name="ps_in")
                            for k in range(KH):
                                nc.tensor.matmul(
                                    ps_in[j][:],
                                    w_cn_sb[:, k, j * 128:(j + 1) * 128],
                                    ht_c[:, k, :],
                                    start=(k == 0), stop=(k == KH - 1),
                                )
                        ps_hn = psp.tile([128, NH], F32, tag="ps")
                        for k in range(KH):
                            nc.tensor.matmul(
                                ps_hn[:],
                                w_hn_sb[:, k, j * 128:(j + 1) * 128],
                                ht_c[:, k, :],
                                start=(k == 0), stop=(k == KH - 1),
                            )
                        hnb = scr.tile([128, NH], F32, tag="hnb")
                        nc.scalar.activation(
                            hnb[:], ps_hn[:], AF.Identity, bias=b_hn_sb[:, j:j + 1]
                        )
                        t1 = scr.tile([128, NH], F32, tag="t1")
                        nc.vector.tensor_mul(t1[:], r_t[:], hnb[:])
                        t2 = scr.tile([128, NH], F32, tag="t2")
                        if t == 0:
                            nc.vector.tensor_add(t2[:], in_t[:], t1[:])
                            n_t = scr.tile([128, NH], F32, tag="n")
                            nc.scalar.activation(n_t[:], t2[:], AF.Tanh)
                        else:
                            nc.vector.tensor_add(t2[:], ps_in[j][:], t1[:])
                            n_t = scr.tile([128, NH], F32, tag="n")
                            nc.scalar.activation(
                                n_t[:], t2[:], AF.Tanh, bias=bin_[:, j:j + 1]
                            )
                        d_t = scr.tile([128, NH], F32, tag="d")
                        nc.vector.tensor_sub(
                            d_t[:], h_own[cur][hf][:, j, :].bitcast(F32), n_t[:]
                        )
                        zd = scr.tile([128, NH], F32, tag="zd")
                        nc.vector.tensor_mul(zd[:], z_t[:], d_t[:])
                        # fp32r-rounding write of the new own h slice
                        nc.vector.tensor_add(
                            h_own[nxt][hf][:, j, :], n_t[:], zd[:]
                        )
                        # bounce own slice straight out to the stage buffer
                        nc.sync.dma_start(
                            hstage[cur][j * 128:(j + 1) * 128, cols],
                            h_own[nxt][hf][:, j, :],
                        )

                # ---- one all-gather per step (both halves) --------------
                nc.gpsimd.collective_compute(
                    "AllGather",
                    mybir.AluOpType.bypass,
                    replica_groups=[list(range(G))],
                    ins=[hstage[cur][:]],
                    outs=[hgath[cur][:]],
                )
                gat = hgath[cur][:].rearrange("(k p) n -> p k n", p=128)
                for hf in (0, 1):
                    cols = slice(hf * NH, (hf + 1) * NH)
                    for k0 in range(0, KH, 4):
                        sl = slice(k0, k0 + 4)
                        nc.sync.dma_start(
                            ht_sb[nxt][hf][:, sl, :], gat[:, sl, cols]
                        )

                # fc output for this step: y[t] = h_{t+1} @ Wfc.T + b_fc
                for hf in (0, 1):
                    cols = slice(hf * NH, (hf + 1) * NH)
                    ps_fc = psp.tile([128, NH], F32, tag="ps")
                    for k in range(KH):
                        nc.tensor.matmul(
                            ps_fc[:],
                            w_fc_sb[:, k, :],
                            ht_sb[nxt][hf][:, k, :],
                            start=(k == 0), stop=(k == KH - 1),
                        )
                    y_sb = scr.tile([128, NH], F32, tag="y")
                    nc.scalar.activation(
                        y_sb[:], ps_fc[:], AF.Identity, bias=b_fc_sb[:, 0:1]
                    )
                    nc.sync.dma_start(y[t, :, cols], y_sb[:])

    nc.compile()
    return nc


def prep_in_maps(inputs, h_0, W_ih, W_hh, b_ih, b_hh, W_fc, b_fc):
    """Host-side sharding/layout prep. Returns per-core in_maps."""
    W_ih64 = np.asarray(W_ih, np.float64)
    W_hh64 = np.asarray(W_hh, np.float64)
    W_fc64 = np.asarray(W_fc, np.float64)
    b_ih = np.asarray(b_ih, np.float32)
    b_hh = np.asarray(b_hh, np.float32)
    b_fc32 = np.asarray(b_fc, np.float32)

    Wc = W_ih64 @ W_fc64                       # [3H, H]
    bias_comb = W_ih64 @ np.asarray(b_fc, np.float64)  # [3H]

    x0_t = round_fp32r(np.asarray(inputs, np.float32).T)
    h0_t = round_fp32r(np.asarray(h_0, np.float32).T)

    in_maps = []
    for c in range(G):
        rs = np.arange(HL * c, HL * (c + 1))
        idx_rz = np.concatenate([rs, H + rs])
        idx_n = 2 * H + rs
        idx_rzn = np.concatenate([idx_rz, idx_n])
        xs = slice(XL * c, XL * (c + 1))

        w_rz_c = round_fp32r((Wc[idx_rz] + W_hh64[idx_rz]).T.astype(np.float32))
        w_cn_c = round_fp32r(Wc[idx_n].T.astype(np.float32))
        w_hn_c = round_fp32r(W_hh64[idx_n].T.astype(np.float32))
        w_fc_c = round_fp32r(W_fc64[xs].T.astype(np.float32))
        w_ih0_c = round_fp32r(W_ih64[idx_rzn].T.astype(np.float32))
        w_hh0_c = round_fp32r(W_hh64[idx_rz].T.astype(np.float32))

        b_rz_c = (b_ih[idx_rz].astype(np.float64)
                  + b_hh[idx_rz] + bias_comb[idx_rz]).astype(np.float32)
        b_rz0_c = b_ih[idx_rz] + b_hh[idx_rz]
        b_in_c = (b_ih[idx_n].astype(np.float64)
                  + bias_comb[idx_n]).astype(np.float32)
        b_in0_c = b_ih[idx_n]
        b_hn_c = b_hh[idx_n]

        in_maps.append({
            "w_rz": np.ascontiguousarray(w_rz_c),
            "w_cn": np.ascontiguousarray(w_cn_c),
            "w_hn": np.ascontiguousarray(w_hn_c),
            "w_fc": np.ascontiguousarray(w_fc_c),
            "w_ih0": np.ascontiguousarray(w_ih0_c),
            "w_hh0": np.ascontiguousarray(w_hh0_c),
            "x0": x0_t,
            "h0": h0_t,
            "h0_own": np.ascontiguousarray(h0_t[HL * c:HL * (c + 1)]),
            "b_rz": np.ascontiguousarray(b_rz_c.reshape(4, 128).T),
            "b_rz0": np.ascontiguousarray(b_rz0_c.reshape(4, 128).T),
            "b_in": np.ascontiguousarray(b_in_c.reshape(2, 128).T),
            "b_in0": np.ascontiguousarray(b_in0_c.reshape(2, 128).T),
            "b_hn": np.ascontiguousarray(b_hn_c.reshape(2, 128).T),
            "b_fc": np.ascontiguousarray(b_fc32[xs].reshape(1, 128).T),
        })
    return in_maps


def assemble_output(results, T: int) -> np.ndarray:
    """Per-core y [T, 128, B] (features x batch) -> [B, T, I], time-reversed."""
    out = np.empty((B, T, I), np.float32)
    for c, res in enumerate(results):
        yc = res["y"]                      # [T, 128, B]
        # reverse time, put batch first
        out[:, :, XL * c:XL * (c + 1)] = yc[::-1].transpose(2, 0, 1)
    return out


_NC_CACHE: dict = {}


def _get_nc(T: int):
    if T not in _NC_CACHE:
        _NC_CACHE[T] = build(T)
    return _NC_CACHE[T]


def kernel(inputs, h_0, W_ih, W_hh, b_ih, b_hh, W_fc, b_fc, seq_len):
    T = int(seq_len)
    nc = _get_nc(T)
    in_maps = prep_in_maps(inputs, h_0, W_ih, W_hh, b_ih, b_hh, W_fc, b_fc)
    res = run_bass_kernel_spmd(nc, in_maps, list(range(G)))
    return assemble_output(res.results, T)

